# revision 13
# baseline (speedup 1.0000x reference)
"""Trainium2 Bass kernel for dual cross-attention (CotSR block).

Problem: two cross-attentions between x1, x2 [B=4, C=512, H=W=64].
  q1 = wq1@x1, k2 = wk2@x2, v2 = wv2@x2 ; att1 = softmax(q1^T k2) over keys
  out1 = x1 + gamma1 * (v2 @ att1^T)   (and symmetrically for out2)

Sharding: 8 independent (batch, direction) jobs -> one per NeuronCore.

v2 design (vs bf16 baseline):
  - PV matmul in fp8 DoubleRow: PT (exp scores) e5m2, VT e4m3, contraction
    256 keys/MM (pairs of 128-key tiles) -> ~1.8x PE throughput on the
    dominant matmul. exp computed with a -4 bias (cancels in softmax
    normalization) to keep e5m2 in range (S in [-12, 11.5]).
  - ST (K^T Q, contraction DQ=64) row-packed 2x via tile_position: even
    key tiles on PE rows 0-63, odd tiles on rows 64-127 concurrently.
    Needs Q and K each present on both partition halves -> QK1 = [Q; K],
    QK2 = [K; Q], produced by col-packed projections (Q cols 0-63 and
    K cols 64-127 of the PE array run concurrently).
  - Rowsum: all-ones [128,2,128] fp8 DoubleRow matmul per key-pair,
    accumulated across all 16 pairs in a dedicated psum bank (every
    partition holds the rowsum: broadcast for free). Zero per-pair DVE
    work -> DVE off the critical path (v2 lesson: each DVE op carries
    ~800ns semaphore/drain tax and the rowsum-evict gated ST psum reuse).
  - PSUM: o_all = one 4-bank [128,2048] tile (4 channel chunks), rs = 1
    bank, ST = 3 single-bank tiles. Epilogue is 3 wide DVE ops total:
    reciprocal_approx_fast + one [128,2048] mul (recip broadcast via
    0-stride AP) + one [128,2048] scalar_tensor_tensor.
"""

import numpy as np

import concourse.bass as bass
import concourse.mybir as mybir
import concourse.tile as tile
from concourse import bacc
import concourse.bass_utils as _bu

# walrus's --enable-ldw-opt=false serializes every LDWEIGHTS with its MATMUL
# (measured 379 ns/MM vs ~215 warm); enable background-weight-buffer overlap.
_orig_run_command = _bu.run_command


def _patched_run_command(argv, **kw):
    argv = ["--enable-ldw-opt=true" if a == "--enable-ldw-opt=false" else a
            for a in argv]
    return _orig_run_command(argv, **kw)


_bu.run_command = _patched_run_command
from concourse.bass_utils import run_bass_kernel_spmd
from concourse._compat import with_exitstack
from contextlib import ExitStack

F32 = mybir.dt.float32
BF16 = mybir.dt.bfloat16
E4 = mybir.dt.float8e4
E5 = mybir.dt.float8e5
AF = mybir.ActivationFunctionType
ALU = mybir.AluOpType
PM = mybir.MatmulPerfMode
ts = bass.ts

B, C, H, W = 4, 512, 64, 64
N = H * W          # 4096
DQ = 64
P = 128
QB = 512           # query block (free dim / psum bank)
NQB = N // QB      # 8 query blocks
NKT = N // P       # 32 key tiles
NPAIR = NKT // 2   # 16 key-tile pairs (DoubleRow contracts 256 keys)
NCC = C // P       # 4 channel chunks
EXP_BIAS = -4.0    # exp(S-4): cancels in softmax; keeps e5m2 in range


@with_exitstack
def _body(ctx: ExitStack, tc: "tile.TileContext", io: dict):
    nc = tc.nc
    xq_d, xkv_d, wq_d, wk_d, wv_d = io["xq"], io["xkv"], io["wq"], io["wk"], io["wv"]
    bq_d, bk_d, bv_d, gamma_d, out_d = io["bq"], io["bk"], io["bv"], io["gamma"], io["out"]

    const = ctx.enter_context(tc.tile_pool(name="const", bufs=1))
    persist = ctx.enter_context(tc.tile_pool(name="persist", bufs=1))
    wpool = ctx.enter_context(tc.tile_pool(name="wpool", bufs=1))
    stage = ctx.enter_context(tc.tile_pool(name="stage", bufs=3))
    ptp = ctx.enter_context(tc.tile_pool(name="ptp", bufs=3))
    dvp = ctx.enter_context(tc.tile_pool(name="dvp", bufs=3))
    # PSUM: pso = o_all [128,2048] (4 banks), pst = 2x [128,1024] (4 banks;
    # ST pair tiles so exp runs as one FD=1024 ACT instruction / proj scratch)
    pso = ctx.enter_context(tc.tile_pool(name="pso", bufs=1, space="PSUM"))
    pst = ctx.enter_context(tc.tile_pool(name="pst", bufs=2, space="PSUM"))

    # ---- constants ----
    ones_pair = const.tile([P, 2, P], E4, tag="ones_pair", name="ones_pair")
    nc.vector.memset(ones_pair, 1.0)
    ones_row_bf = const.tile([1, P], BF16, tag="ones_row_bf", name="ones_row_bf")
    nc.vector.memset(ones_row_bf, 1.0)
    expb = const.tile([P, 1], F32, tag="expb", name="expb")
    nc.vector.memset(expb, EXP_BIAS)

    # ---- small inputs ----
    # bias1 = [bq; bk] for QK1 = [Q; K], bias2 = [bk; bq] for QK2 = [K; Q]
    bias1 = const.tile([P, 1], F32, tag="bias1", name="bias1")
    nc.sync.dma_start(bias1[0:DQ, :], bq_d)
    nc.sync.dma_start(bias1[DQ:P, :], bk_d)
    bias2 = const.tile([P, 1], F32, tag="bias2", name="bias2")
    nc.sync.dma_start(bias2[0:DQ, :], bk_d)
    nc.sync.dma_start(bias2[DQ:P, :], bq_d)
    bv_sb = const.tile([1, C], F32, tag="bv", name="bv_sb")
    nc.sync.dma_start(bv_sb, bv_d)
    bv_bf = const.tile([1, C], BF16, tag="bvbf", name="bv_bf")
    nc.vector.tensor_copy(bv_bf, bv_sb)
    gamma_b = const.tile([P, 1], F32, tag="gamma_b", name="gamma_b")
    nc.sync.dma_start(gamma_b, gamma_d)

    # bv broadcast to all partitions once: [128, C] f32 (via rank-1 matmul)
    bvb_ps = pst.tile([P, QB], F32, tag="st", name="bvb_ps")
    nc.tensor.matmul(bvb_ps, ones_row_bf, bv_bf, start=True, stop=True)
    bv_bcast = const.tile([P, C], F32, tag="bv_bcast", name="bv_bcast")
    nc.vector.tensor_copy(bv_bcast, bvb_ps)

    # ---- weights arrive PRE-TRANSPOSED from host: wq_d/wk_d are [C, DQ],
    # wv_d is [C(c'), C(c)] = wv.T ; fp8 channel-chunk-pair layout so the
    # projections run DoubleRow (256-channel contraction per MM) ----
    wqT_pair, wkT_pair, wvT_pair = [], [], []
    for p in range(2):
        wq8 = wpool.tile([P, 2, DQ], E4, tag=f"wqT{p}", name=f"wqT{p}")
        wk8 = wpool.tile([P, 2, DQ], E4, tag=f"wkT{p}", name=f"wkT{p}")
        wv8 = wpool.tile([P, 2, C], E4, tag=f"wvT{p}", name=f"wvT{p}")
        wqT_pair.append(wq8)
        wkT_pair.append(wk8)
        wvT_pair.append(wv8)
        for j in range(2):
            cc = 2 * p + j
            for (src_d, dst) in ((wq_d, wq8), (wk_d, wk8)):
                wst = stage.tile([P, DQ], F32, tag="w_stage", name="w_st")
                nc.sync.dma_start(wst, src_d[ts(cc, P), :])
                nc.vector.tensor_copy(dst[:, j, :], wst)
            wst2 = stage.tile([P, C], F32, tag="w_stage2", name="w_st2")
            nc.sync.dma_start(wst2, wv_d[ts(cc, P), :])
            nc.vector.tensor_copy(wv8[:, j, :], wst2)

    # ---- xq/xkv resident fp8e4 in channel-chunk-pair layout;
    # gpsimd DMA casts f32->fp8 in flight ----
    xq_f8 = [persist.tile([P, 2, N], E4, tag=f"xq{p}", name=f"xq_f8{p}")
             for p in range(2)]
    xkv_f8 = [persist.tile([P, 2, N], E4, tag=f"xkv{p}", name=f"xkv_f8{p}")
              for p in range(2)]

    def emit_x_load(h):  # 512-col half-pieces so the first proj starts earlier
        for g in range(2):
            for p in range(2):
                for j in range(2):
                    nc.gpsimd.dma_start(xkv_f8[p][:, j, ts(2 * h + g, QB)],
                                        xkv_d[ts(2 * p + j, P), ts(2 * h + g, QB)])
            for p in range(2):
                for j in range(2):
                    nc.gpsimd.dma_start(xq_f8[p][:, j, ts(2 * h + g, QB)],
                                        xq_d[ts(2 * p + j, P), ts(2 * h + g, QB)])

    # ---- persistent projection outputs ----
    # QK1 = [Q(0:64); K(64:128)], QK2 = [K(0:64); Q(64:128)], both [128, N]
    QK1_sb = persist.tile([P, N], BF16, tag="QK1", name="QK1_sb")
    QK2_sb = persist.tile([P, N], BF16, tag="QK2", name="QK2_sb")
    # VT pairs: [128 keys, 2(ktile of pair), 512 ch] fp8e4 per pair
    VT_pair = [persist.tile([P, 2, C], E4, tag=f"VT{T}", name=f"VT{T}")
               for T in range(NPAIR)]

    # ---- projections (fp8 DoubleRow: 256-channel contraction per MM) ----
    # Q and K each computed once at partitions 0-63 (DoubleRow forbids a
    # col-offset dst), biased on DVE, then replicated to partitions 64-127
    # by SBUF->SBUF DMA: QK1 = [Q; K], QK2 = [K; Q].
    def emit_qk_proj(nb):
        qk_ps = pst.tile([P, 2 * QB], F32, tag="st", name="qk_ps")
        for p in range(2):
            nc.tensor.matmul(qk_ps[0:DQ, 0:QB], wqT_pair[p],
                             xq_f8[p][:, :, ts(nb, QB)],
                             start=(p == 0), stop=(p == 1),
                             perf_mode=PM.DoubleRow)
            nc.tensor.matmul(qk_ps[0:DQ, QB:2 * QB], wkT_pair[p],
                             xkv_f8[p][:, :, ts(nb, QB)],
                             start=(p == 0), stop=(p == 1),
                             perf_mode=PM.DoubleRow)
        nc.vector.tensor_scalar(QK1_sb[0:DQ, ts(nb, QB)], qk_ps[0:DQ, 0:QB],
                                bias1[0:DQ, :], None, op0=ALU.add)
        nc.vector.tensor_scalar(QK2_sb[0:DQ, ts(nb, QB)], qk_ps[0:DQ, QB:2 * QB],
                                bias2[0:DQ, :], None, op0=ALU.add)
        nc.sync.dma_start(QK2_sb[DQ:P, ts(nb, QB)], QK1_sb[0:DQ, ts(nb, QB)])
        nc.sync.dma_start(QK1_sb[DQ:P, ts(nb, QB)], QK2_sb[0:DQ, ts(nb, QB)])

    def emit_v_proj(T):
        vp = pst.tile([P, 2 * QB], F32, tag="st", name="v_ps")
        for j in range(2):
            t = 2 * T + j
            for p in range(2):
                nc.tensor.matmul(vp[:, ts(j, QB)], xkv_f8[p][:, :, ts(t, P)],
                                 wvT_pair[p], start=(p == 0), stop=(p == 1),
                                 perf_mode=PM.DoubleRow)
        # one fused eviction for both key tiles of the pair
        nc.vector.tensor_add(
            VT_pair[T][:, :, :],
            vp[:, :].rearrange("p (a b) -> p a b", a=2),
            bv_bcast[:, :].rearrange("p (a b) -> p a b", a=1)
                          .broadcast_to((P, 2, C)))

    # ---- attention emission (interleaved with proj for qb0) ----
    qstate = {}

    def emit_attn_pair(qb, T):
        o_all, acc_rs = qstate[qb]
        # even key tile on PE rows 0-63, odd on rows 64-127 (concurrent)
        stp = pst.tile([P, 2 * QB], F32, tag="st", name="st_p")
        nc.tensor.matmul(stp[:, 0:QB], QK2_sb[0:DQ, ts(2 * T, P)],
                         QK1_sb[0:DQ, ts(qb, QB)], start=True, stop=True)
        nc.tensor.matmul(stp[:, QB:2 * QB], QK1_sb[DQ:P, ts(2 * T + 1, P)],
                         QK2_sb[DQ:P, ts(qb, QB)], start=True, stop=True)
        pt = ptp.tile([P, 2, QB], E5, tag="pt", name="pt_sb", bufs=8)
        nc.scalar.activation(pt[:, :, :].rearrange("p a b -> p (a b)"),
                             stp[:, :], AF.Exp, bias=expb)
        for cc in range(NCC):
            nc.tensor.matmul(o_all[:, ts(cc, QB)], VT_pair[T][:, :, ts(cc, P)],
                             pt[:, :, :], start=(T == 0),
                             stop=(T == NPAIR - 1), perf_mode=PM.DoubleRow,
                             skip_group_check=True)
        # rowsum of this pair -> recycled first half of its ST psum tile
        # (all-ones stationary: every partition holds the rowsum), then
        # accumulated into SBUF by DVE well off the critical path
        nc.tensor.matmul(stp[:, 0:QB], ones_pair, pt[:, :, :],
                         start=True, stop=True, perf_mode=PM.DoubleRow,
                         skip_group_check=True)
        if T == 0:
            nc.vector.tensor_copy(acc_rs, stp[:, 0:QB])
        else:
            nc.vector.tensor_add(acc_rs, acc_rs, stp[:, 0:QB])

    def emit_attn_begin(qb):
        qstate[qb] = (pso.tile([P, NCC * QB], F32, tag="oall", name="o_all"),
                      dvp.tile([P, QB], F32, tag=f"accrs{qb % 2}",
                               name="acc_rs", bufs=1))

    def emit_attn_end(qb):
        # epilogue: out = x + gamma * O / rowsum  (3 wide DVE ops)
        o_all, acc_rs = qstate.pop(qb)
        recip_b = dvp.tile([P, QB], F32, tag=f"recip{qb % 2}", name="recip_b",
                           bufs=1)
        nc.vector.reciprocal_approx_fast(out=recip_b[:, :], in_=acc_rs[:, :])
        xr4 = stage.tile([P, NCC * QB], F32, tag="xres", name="x_res", bufs=2)
        for cc in range(NCC):
            nc.sync.dma_start(xr4[:, ts(cc, QB)], xq_d[ts(cc, P), ts(qb, QB)])
        t1 = dvp.tile([P, NCC * QB], F32, tag="t1", name="t1", bufs=2)
        nc.vector.tensor_mul(
            t1[:, :].rearrange("p (a b) -> p a b", a=NCC),
            o_all[:, :].rearrange("p (a b) -> p a b", a=NCC),
            recip_b[:, :].rearrange("p (a b) -> p a b", a=1)
                         .broadcast_to((P, NCC, QB)))
        og = dvp.tile([P, NCC * QB], F32, tag="og", name="og", bufs=2)
        nc.vector.scalar_tensor_tensor(og, t1, gamma_b, xr4,
                                       op0=ALU.mult, op1=ALU.add)
        for cc in range(NCC):
            nc.sync.dma_start(out_d[ts(cc, P), ts(qb, QB)], og[:, ts(cc, QB)])

    # lead-in: x pieces + projections, with qb0's attention interleaved so
    # the PE has attention work as soon as deps allow
    emit_attn_begin(0)
    for h in range(4):
        emit_x_load(h)
        emit_qk_proj(2 * h)
        emit_qk_proj(2 * h + 1)
        for T in range(4 * h, 4 * h + 4):
            emit_v_proj(T)
        if h >= 1:
            for T in range(4 * (h - 1), 4 * h):
                emit_attn_pair(0, T)
    for T in range(12, NPAIR):
        emit_attn_pair(0, T)
    emit_attn_end(0)

    for qb in range(1, NQB):
        emit_attn_begin(qb)
        for T in range(NPAIR):
            emit_attn_pair(qb, T)
        emit_attn_end(qb)


_NC_CACHE = {}


def _fuse_ldweights(nc):
    """Re-fuse Tile's split LDWEIGHTS+MATMUL pairs into self-loading matmuls
    so walrus's ldw-opt (background weight buffer) can overlap weight loads
    with in-flight matmuls."""
    for b in nc.m.functions[0].blocks:
        out = []
        pending = None
        for i in b.instructions:
            tn = type(i).__name__
            if tn == "InstLdweights":
                assert pending is None, "back-to-back ldweights"
                pending = i
                continue
            if tn == "InstMatmult" and pending is not None:
                i.ldweights = True
                si = pending.sync_info
                if si is not None and (si.on_wait or si.on_update):
                    if i.sync_info is None:
                        i.sync_info = mybir.SyncInfo(on_wait=[], on_update=[])
                    i.sync_info.on_wait = list(si.on_wait) + list(i.sync_info.on_wait)
                    i.sync_info.on_update = (list(si.on_update)
                                             + list(i.sync_info.on_update))
                pending = None
            out.append(i)
        assert pending is None, "trailing ldweights without matmul"
        b.instructions[:] = out


def _build():
    if "nc" in _NC_CACHE:
        return _NC_CACHE["nc"]
    nc = bacc.Bacc("TRN2", target_bir_lowering=False, debug=False, num_devices=8)
    io = {
        "xq": nc.dram_tensor("xq", [C, N], F32, kind="ExternalInput").ap(),
        "xkv": nc.dram_tensor("xkv", [C, N], F32, kind="ExternalInput").ap(),
        "wq": nc.dram_tensor("wq", [C, DQ], F32, kind="ExternalInput").ap(),
        "wk": nc.dram_tensor("wk", [C, DQ], F32, kind="ExternalInput").ap(),
        "wv": nc.dram_tensor("wv", [C, C], F32, kind="ExternalInput").ap(),
        "bq": nc.dram_tensor("bq", [DQ, 1], F32, kind="ExternalInput").ap(),
        "bk": nc.dram_tensor("bk", [DQ, 1], F32, kind="ExternalInput").ap(),
        "bv": nc.dram_tensor("bv", [1, C], F32, kind="ExternalInput").ap(),
        "gamma": nc.dram_tensor("gamma", [128, 1], F32, kind="ExternalInput").ap(),
        "out": nc.dram_tensor("out", [C, N], F32, kind="ExternalOutput").ap(),
    }
    with tile.TileContext(nc) as tc:
        _body(tc, io)
    _fuse_ldweights(nc)
    nc.compile()
    _NC_CACHE["nc"] = nc
    return nc


def make_in_maps(x1, x2, wq1, bq1, wk1, bk1, wv1, bv1,
                 wq2, bq2, wk2, bk2, wv2, bv2, gamma1, gamma2):
    """Returns the 8 per-core input dicts. Cores 0-3: out1[b]; 4-7: out2[b]."""
    f = np.ascontiguousarray
    x1f = np.asarray(x1, np.float32).reshape(B, C, N)
    x2f = np.asarray(x2, np.float32).reshape(B, C, N)
    maps = []
    for b in range(B):
        maps.append({
            "xq": f(x1f[b]), "xkv": f(x2f[b]),
            "wq": f(np.asarray(wq1, np.float32).T),
            "wk": f(np.asarray(wk2, np.float32).T),
            "wv": f(np.asarray(wv2, np.float32).T),
            "bq": f(np.asarray(bq1, np.float32).reshape(DQ, 1)),
            "bk": f(np.asarray(bk2, np.float32).reshape(DQ, 1)),
            "bv": f(np.asarray(bv2, np.float32).reshape(1, C)),
            "gamma": f(np.tile(np.asarray(gamma1, np.float32).reshape(1, 1), (128, 1))),
        })
    for b in range(B):
        maps.append({
            "xq": f(x2f[b]), "xkv": f(x1f[b]),
            "wq": f(np.asarray(wq2, np.float32).T),
            "wk": f(np.asarray(wk1, np.float32).T),
            "wv": f(np.asarray(wv1, np.float32).T),
            "bq": f(np.asarray(bq2, np.float32).reshape(DQ, 1)),
            "bk": f(np.asarray(bk1, np.float32).reshape(DQ, 1)),
            "bv": f(np.asarray(bv1, np.float32).reshape(1, C)),
            "gamma": f(np.tile(np.asarray(gamma2, np.float32).reshape(1, 1), (128, 1))),
        })
    return maps


def kernel(**inputs):
    nc = _build()
    in_maps = make_in_maps(**inputs)
    res = run_bass_kernel_spmd(nc, in_maps, list(range(8))).results
    out1 = np.stack([res[b]["out"].reshape(C, H, W) for b in range(B)])
    out2 = np.stack([res[B + b]["out"].reshape(C, H, W) for b in range(B)])
    return out1, out2


# revision 15
# speedup vs baseline: 1.2476x; 1.2476x over previous
"""Trainium2 Bass kernel for dual cross-attention (CotSR block).

Problem: two cross-attentions between x1, x2 [B=4, C=512, H=W=64].
  q1 = wq1@x1, k2 = wk2@x2, v2 = wv2@x2 ; att1 = softmax(q1^T k2) over keys
  out1 = x1 + gamma1 * (v2 @ att1^T)   (and symmetrically for out2)

Sharding: 8 independent (batch, direction) jobs -> one per NeuronCore.

v2 design (vs bf16 baseline):
  - PV matmul in fp8 DoubleRow: PT (exp scores) e5m2, VT e4m3, contraction
    256 keys/MM (pairs of 128-key tiles) -> ~1.8x PE throughput on the
    dominant matmul. exp computed with a -4 bias (cancels in softmax
    normalization) to keep e5m2 in range (S in [-12, 11.5]).
  - ST (K^T Q, contraction DQ=64) row-packed 2x via tile_position: even
    key tiles on PE rows 0-63, odd tiles on rows 64-127 concurrently.
    Needs Q and K each present on both partition halves -> QK1 = [Q; K],
    QK2 = [K; Q], produced by col-packed projections (Q cols 0-63 and
    K cols 64-127 of the PE array run concurrently).
  - Rowsum: all-ones [128,2,128] fp8 DoubleRow matmul per key-pair,
    accumulated across all 16 pairs in a dedicated psum bank (every
    partition holds the rowsum: broadcast for free). Zero per-pair DVE
    work -> DVE off the critical path (v2 lesson: each DVE op carries
    ~800ns semaphore/drain tax and the rowsum-evict gated ST psum reuse).
  - PSUM: o_all = one 4-bank [128,2048] tile (4 channel chunks), rs = 1
    bank, ST = 3 single-bank tiles. Epilogue is 3 wide DVE ops total:
    reciprocal_approx_fast + one [128,2048] mul (recip broadcast via
    0-stride AP) + one [128,2048] scalar_tensor_tensor.
"""

import numpy as np

import concourse.bass as bass
import concourse.mybir as mybir
import concourse.tile as tile
from concourse import bacc
import concourse.bass_utils as _bu

# walrus's --enable-ldw-opt=false serializes every LDWEIGHTS with its MATMUL
# (measured 379 ns/MM vs ~215 warm); enable background-weight-buffer overlap.
_orig_run_command = _bu.run_command


def _patched_run_command(argv, **kw):
    argv = ["--enable-ldw-opt=true" if a == "--enable-ldw-opt=false" else a
            for a in argv]
    return _orig_run_command(argv, **kw)


_bu.run_command = _patched_run_command
from concourse.bass_utils import run_bass_kernel_spmd
from concourse._compat import with_exitstack
from contextlib import ExitStack

F32 = mybir.dt.float32
BF16 = mybir.dt.bfloat16
E4 = mybir.dt.float8e4
E5 = mybir.dt.float8e5
AF = mybir.ActivationFunctionType
ALU = mybir.AluOpType
PM = mybir.MatmulPerfMode
ts = bass.ts

B, C, H, W = 4, 512, 64, 64
N = H * W          # 4096
DQ = 64
P = 128
QB = 512           # query block (free dim / psum bank)
NQB = N // QB      # 8 query blocks
NKT = N // P       # 32 key tiles
NPAIR = NKT // 2   # 16 key-tile pairs (DoubleRow contracts 256 keys)
NCC = C // P       # 4 channel chunks
EXP_BIAS = -4.0    # exp(S-4): cancels in softmax; keeps e5m2 in range


@with_exitstack
def _body(ctx: ExitStack, tc: "tile.TileContext", io: dict):
    nc = tc.nc
    xq_d, xkv_d, wq_d, wk_d, wv_d = io["xq"], io["xkv"], io["wq"], io["wk"], io["wv"]
    bq_d, bk_d, bv_d, gamma_d, out_d = io["bq"], io["bk"], io["bv"], io["gamma"], io["out"]

    const = ctx.enter_context(tc.tile_pool(name="const", bufs=1))
    persist = ctx.enter_context(tc.tile_pool(name="persist", bufs=1))
    wpool = ctx.enter_context(tc.tile_pool(name="wpool", bufs=1))
    stage = ctx.enter_context(tc.tile_pool(name="stage", bufs=3))
    ptp = ctx.enter_context(tc.tile_pool(name="ptp", bufs=3))
    dvp = ctx.enter_context(tc.tile_pool(name="dvp", bufs=3))
    # PSUM: pso = o_all [128,2048] (4 banks), prs = rowsum (1 bank),
    # pst = 3x [128,512] (ST tiles / proj scratch)
    pso = ctx.enter_context(tc.tile_pool(name="pso", bufs=1, space="PSUM"))
    prs = ctx.enter_context(tc.tile_pool(name="prs", bufs=1, space="PSUM"))
    pst = ctx.enter_context(tc.tile_pool(name="pst", bufs=3, space="PSUM"))

    # ---- constants ----
    ones_pair = const.tile([P, 2, P], E4, tag="ones_pair", name="ones_pair")
    nc.vector.memset(ones_pair, 1.0)
    ones_row_bf = const.tile([1, P], BF16, tag="ones_row_bf", name="ones_row_bf")
    nc.vector.memset(ones_row_bf, 1.0)
    expb = const.tile([P, 1], F32, tag="expb", name="expb")
    nc.vector.memset(expb, EXP_BIAS)

    # ---- small inputs ----
    # bias1 = [bq; bk] for QK1 = [Q; K], bias2 = [bk; bq] for QK2 = [K; Q]
    bias1 = const.tile([P, 1], F32, tag="bias1", name="bias1")
    nc.sync.dma_start(bias1[0:DQ, :], bq_d)
    nc.sync.dma_start(bias1[DQ:P, :], bk_d)
    bias2 = const.tile([P, 1], F32, tag="bias2", name="bias2")
    nc.sync.dma_start(bias2[0:DQ, :], bk_d)
    nc.sync.dma_start(bias2[DQ:P, :], bq_d)
    bv_sb = const.tile([1, C], F32, tag="bv", name="bv_sb")
    nc.sync.dma_start(bv_sb, bv_d)
    bv_bf = const.tile([1, C], BF16, tag="bvbf", name="bv_bf")
    nc.vector.tensor_copy(bv_bf, bv_sb)
    gamma_b = const.tile([P, 1], F32, tag="gamma_b", name="gamma_b")
    nc.sync.dma_start(gamma_b, gamma_d)

    # bv broadcast to all partitions once: [128, C] f32 (via rank-1 matmul)
    bvb_ps = pst.tile([P, QB], F32, tag="st", name="bvb_ps")
    nc.tensor.matmul(bvb_ps, ones_row_bf, bv_bf, start=True, stop=True)
    bv_bcast = const.tile([P, C], F32, tag="bv_bcast", name="bv_bcast")
    nc.vector.tensor_copy(bv_bcast, bvb_ps)

    # ---- weights arrive PRE-TRANSPOSED from host: wq_d/wk_d are [C, DQ],
    # wv_d is [C(c'), C(c)] = wv.T ; fp8 channel-chunk-pair layout so the
    # projections run DoubleRow (256-channel contraction per MM) ----
    wqT_pair, wkT_pair, wvT_pair = [], [], []
    for p in range(2):
        wq8 = wpool.tile([P, 2, DQ], E4, tag=f"wqT{p}", name=f"wqT{p}")
        wk8 = wpool.tile([P, 2, DQ], E4, tag=f"wkT{p}", name=f"wkT{p}")
        wv8 = wpool.tile([P, 2, C], E4, tag=f"wvT{p}", name=f"wvT{p}")
        wqT_pair.append(wq8)
        wkT_pair.append(wk8)
        wvT_pair.append(wv8)
        for j in range(2):
            cc = 2 * p + j
            for (src_d, dst) in ((wq_d, wq8), (wk_d, wk8)):
                wst = stage.tile([P, DQ], F32, tag="w_stage", name="w_st")
                nc.sync.dma_start(wst, src_d[ts(cc, P), :])
                nc.vector.tensor_copy(dst[:, j, :], wst)
            wst2 = stage.tile([P, C], F32, tag="w_stage2", name="w_st2")
            nc.sync.dma_start(wst2, wv_d[ts(cc, P), :])
            nc.vector.tensor_copy(wv8[:, j, :], wst2)

    # ---- xq/xkv resident fp8e4 in channel-chunk-pair layout;
    # gpsimd DMA casts f32->fp8 in flight ----
    xq_f8 = [persist.tile([P, 2, N], E4, tag=f"xq{p}", name=f"xq_f8{p}")
             for p in range(2)]
    xkv_f8 = [persist.tile([P, 2, N], E4, tag=f"xkv{p}", name=f"xkv_f8{p}")
              for p in range(2)]

    def emit_x_load(h):  # 512-col half-pieces so the first proj starts earlier
        for g in range(2):
            for p in range(2):
                for j in range(2):
                    nc.gpsimd.dma_start(xkv_f8[p][:, j, ts(2 * h + g, QB)],
                                        xkv_d[ts(2 * p + j, P), ts(2 * h + g, QB)])
            for p in range(2):
                for j in range(2):
                    nc.gpsimd.dma_start(xq_f8[p][:, j, ts(2 * h + g, QB)],
                                        xq_d[ts(2 * p + j, P), ts(2 * h + g, QB)])

    # ---- persistent projection outputs ----
    # QK1 = [Q(0:64); K(64:128)], QK2 = [K(0:64); Q(64:128)], both [128, N]
    QK1_sb = persist.tile([P, N], BF16, tag="QK1", name="QK1_sb")
    QK2_sb = persist.tile([P, N], BF16, tag="QK2", name="QK2_sb")
    # VT pairs: [128 keys, 2(ktile of pair), 512 ch] fp8e4 per pair
    VT_pair = [persist.tile([P, 2, C], E4, tag=f"VT{T}", name=f"VT{T}")
               for T in range(NPAIR)]

    # ---- projections (fp8 DoubleRow: 256-channel contraction per MM) ----
    # Q and K each computed once at partitions 0-63 (DoubleRow forbids a
    # col-offset dst), biased on DVE, then replicated to partitions 64-127
    # by SBUF->SBUF DMA: QK1 = [Q; K], QK2 = [K; Q].
    def emit_qk_proj(nb):
        q_ps = pst.tile([P, QB], F32, tag="st", name="q_ps")
        k_ps = pst.tile([P, QB], F32, tag="st", name="k_ps")
        for p in range(2):
            nc.tensor.matmul(q_ps[0:DQ, :], wqT_pair[p],
                             xq_f8[p][:, :, ts(nb, QB)],
                             start=(p == 0), stop=(p == 1),
                             perf_mode=PM.DoubleRow)
            nc.tensor.matmul(k_ps[0:DQ, :], wkT_pair[p],
                             xkv_f8[p][:, :, ts(nb, QB)],
                             start=(p == 0), stop=(p == 1),
                             perf_mode=PM.DoubleRow)
        nc.vector.tensor_scalar(QK1_sb[0:DQ, ts(nb, QB)], q_ps[0:DQ, :],
                                bias1[0:DQ, :], None, op0=ALU.add)
        nc.vector.tensor_scalar(QK2_sb[0:DQ, ts(nb, QB)], k_ps[0:DQ, :],
                                bias2[0:DQ, :], None, op0=ALU.add)
        nc.sync.dma_start(QK2_sb[DQ:P, ts(nb, QB)], QK1_sb[0:DQ, ts(nb, QB)])
        nc.sync.dma_start(QK1_sb[DQ:P, ts(nb, QB)], QK2_sb[0:DQ, ts(nb, QB)])

    def emit_v_proj(T):
        for j in range(2):
            t = 2 * T + j
            vp = pst.tile([P, QB], F32, tag="st", name="v_ps")
            for p in range(2):
                nc.tensor.matmul(vp, xkv_f8[p][:, :, ts(t, P)], wvT_pair[p],
                                 start=(p == 0), stop=(p == 1),
                                 perf_mode=PM.DoubleRow)
            nc.vector.tensor_add(VT_pair[T][:, j, :], vp, bv_bcast)

    # ---- attention emission (interleaved with proj for qb0) ----
    qstate = {}

    def emit_attn_pair(qb, T):
        o_all, rs_ps = qstate[qb]
        # even key tile on PE rows 0-63, odd on rows 64-127 (concurrent)
        sta = pst.tile([P, QB], F32, tag="st", name="st_a")
        stb = pst.tile([P, QB], F32, tag="st", name="st_b")
        nc.tensor.matmul(sta, QK2_sb[0:DQ, ts(2 * T, P)],
                         QK1_sb[0:DQ, ts(qb, QB)], start=True, stop=True)
        nc.tensor.matmul(stb, QK1_sb[DQ:P, ts(2 * T + 1, P)],
                         QK2_sb[DQ:P, ts(qb, QB)], start=True, stop=True)
        pt = ptp.tile([P, 2, QB], E5, tag="pt", name="pt_sb", bufs=8)
        nc.scalar.activation(pt[:, 0, :], sta, AF.Exp, bias=expb)
        nc.scalar.activation(pt[:, 1, :], stb, AF.Exp, bias=expb)
        for cc in range(NCC):
            nc.tensor.matmul(o_all[:, ts(cc, QB)], VT_pair[T][:, :, ts(cc, P)],
                             pt[:, :, :], start=(T == 0),
                             stop=(T == NPAIR - 1), perf_mode=PM.DoubleRow,
                             skip_group_check=True)
        # rowsum of this pair accumulates in its own psum bank; the
        # all-ones stationary makes every partition hold the rowsum
        nc.tensor.matmul(rs_ps, ones_pair, pt[:, :, :],
                         start=(T == 0), stop=(T == NPAIR - 1),
                         perf_mode=PM.DoubleRow)

    def emit_attn_begin(qb):
        qstate[qb] = (pso.tile([P, NCC * QB], F32, tag="oall", name="o_all"),
                      prs.tile([P, QB], F32, tag="rs", name="rs_ps"))

    def emit_attn_end(qb):
        # epilogue: out = x + gamma * O / rowsum  (3 wide DVE ops)
        o_all, rs_ps = qstate.pop(qb)
        recip_b = dvp.tile([P, QB], F32, tag=f"recip{qb % 2}", name="recip_b",
                           bufs=1)
        nc.vector.reciprocal_approx_fast(out=recip_b[:, :], in_=rs_ps[:, :])
        xr4 = stage.tile([P, NCC * QB], F32, tag="xres", name="x_res", bufs=2)
        for cc in range(NCC):
            nc.sync.dma_start(xr4[:, ts(cc, QB)], xq_d[ts(cc, P), ts(qb, QB)])
        t1 = dvp.tile([P, NCC * QB], F32, tag="t1", name="t1", bufs=2)
        nc.vector.tensor_mul(
            t1[:, :].rearrange("p (a b) -> p a b", a=NCC),
            o_all[:, :].rearrange("p (a b) -> p a b", a=NCC),
            recip_b[:, :].rearrange("p (a b) -> p a b", a=1)
                         .broadcast_to((P, NCC, QB)))
        og = dvp.tile([P, NCC * QB], F32, tag="og", name="og", bufs=2)
        nc.vector.scalar_tensor_tensor(og, t1, gamma_b, xr4,
                                       op0=ALU.mult, op1=ALU.add)
        for cc in range(NCC):
            nc.sync.dma_start(out_d[ts(cc, P), ts(qb, QB)], og[:, ts(cc, QB)])

    # lead-in: x pieces + projections, with qb0's attention interleaved so
    # the PE has attention work as soon as deps allow
    emit_attn_begin(0)
    for h in range(4):
        emit_x_load(h)
        emit_qk_proj(2 * h)
        emit_qk_proj(2 * h + 1)
        for T in range(4 * h, 4 * h + 4):
            emit_v_proj(T)
        if h >= 1:
            for T in range(4 * (h - 1), 4 * h):
                emit_attn_pair(0, T)
    for T in range(12, NPAIR):
        emit_attn_pair(0, T)
    emit_attn_end(0)

    for qb in range(1, NQB):
        emit_attn_begin(qb)
        for T in range(NPAIR):
            emit_attn_pair(qb, T)
        emit_attn_end(qb)


_NC_CACHE = {}


def _fuse_ldweights(nc):
    """Re-fuse Tile's split LDWEIGHTS+MATMUL pairs into self-loading matmuls
    so walrus's ldw-opt (background weight buffer) can overlap weight loads
    with in-flight matmuls."""
    for b in nc.m.functions[0].blocks:
        out = []
        pending = None
        for i in b.instructions:
            tn = type(i).__name__
            if tn == "InstLdweights":
                assert pending is None, "back-to-back ldweights"
                pending = i
                continue
            if tn == "InstMatmult" and pending is not None:
                i.ldweights = True
                si = pending.sync_info
                if si is not None and (si.on_wait or si.on_update):
                    if i.sync_info is None:
                        i.sync_info = mybir.SyncInfo(on_wait=[], on_update=[])
                    i.sync_info.on_wait = list(si.on_wait) + list(i.sync_info.on_wait)
                    i.sync_info.on_update = (list(si.on_update)
                                             + list(i.sync_info.on_update))
                pending = None
            out.append(i)
        assert pending is None, "trailing ldweights without matmul"
        b.instructions[:] = out


def _build():
    if "nc" in _NC_CACHE:
        return _NC_CACHE["nc"]
    nc = bacc.Bacc("TRN2", target_bir_lowering=False, debug=False, num_devices=8)
    io = {
        "xq": nc.dram_tensor("xq", [C, N], F32, kind="ExternalInput").ap(),
        "xkv": nc.dram_tensor("xkv", [C, N], F32, kind="ExternalInput").ap(),
        "wq": nc.dram_tensor("wq", [C, DQ], F32, kind="ExternalInput").ap(),
        "wk": nc.dram_tensor("wk", [C, DQ], F32, kind="ExternalInput").ap(),
        "wv": nc.dram_tensor("wv", [C, C], F32, kind="ExternalInput").ap(),
        "bq": nc.dram_tensor("bq", [DQ, 1], F32, kind="ExternalInput").ap(),
        "bk": nc.dram_tensor("bk", [DQ, 1], F32, kind="ExternalInput").ap(),
        "bv": nc.dram_tensor("bv", [1, C], F32, kind="ExternalInput").ap(),
        "gamma": nc.dram_tensor("gamma", [128, 1], F32, kind="ExternalInput").ap(),
        "out": nc.dram_tensor("out", [C, N], F32, kind="ExternalOutput").ap(),
    }
    with tile.TileContext(nc) as tc:
        _body(tc, io)
    _fuse_ldweights(nc)
    nc.compile()
    _NC_CACHE["nc"] = nc
    return nc


def make_in_maps(x1, x2, wq1, bq1, wk1, bk1, wv1, bv1,
                 wq2, bq2, wk2, bk2, wv2, bv2, gamma1, gamma2):
    """Returns the 8 per-core input dicts. Cores 0-3: out1[b]; 4-7: out2[b]."""
    f = np.ascontiguousarray
    x1f = np.asarray(x1, np.float32).reshape(B, C, N)
    x2f = np.asarray(x2, np.float32).reshape(B, C, N)
    maps = []
    for b in range(B):
        maps.append({
            "xq": f(x1f[b]), "xkv": f(x2f[b]),
            "wq": f(np.asarray(wq1, np.float32).T),
            "wk": f(np.asarray(wk2, np.float32).T),
            "wv": f(np.asarray(wv2, np.float32).T),
            "bq": f(np.asarray(bq1, np.float32).reshape(DQ, 1)),
            "bk": f(np.asarray(bk2, np.float32).reshape(DQ, 1)),
            "bv": f(np.asarray(bv2, np.float32).reshape(1, C)),
            "gamma": f(np.tile(np.asarray(gamma1, np.float32).reshape(1, 1), (128, 1))),
        })
    for b in range(B):
        maps.append({
            "xq": f(x2f[b]), "xkv": f(x1f[b]),
            "wq": f(np.asarray(wq2, np.float32).T),
            "wk": f(np.asarray(wk1, np.float32).T),
            "wv": f(np.asarray(wv1, np.float32).T),
            "bq": f(np.asarray(bq2, np.float32).reshape(DQ, 1)),
            "bk": f(np.asarray(bk1, np.float32).reshape(DQ, 1)),
            "bv": f(np.asarray(bv1, np.float32).reshape(1, C)),
            "gamma": f(np.tile(np.asarray(gamma2, np.float32).reshape(1, 1), (128, 1))),
        })
    return maps


def kernel(**inputs):
    nc = _build()
    in_maps = make_in_maps(**inputs)
    res = run_bass_kernel_spmd(nc, in_maps, list(range(8))).results
    out1 = np.stack([res[b]["out"].reshape(C, H, W) for b in range(B)])
    out2 = np.stack([res[B + b]["out"].reshape(C, H, W) for b in range(B)])
    return out1, out2
